# revision 24
# baseline (speedup 1.0000x reference)
"""MoE BatchedExperts kernel for 8 trn2 NeuronCores.

Strategy: expert parallelism with host-side top-k dispatch. Each token has
exactly TOP_K nonzero routing weights, so core e only processes the tokens
routed to expert e (~N*K/E of them) instead of all N — 4x less compute than
the dense reference formulation, identical math (zero-score tokens
contribute zero).

Per core e (tokens gathered+transposed on host to xT [D, T]):
  hT = gelu(mm1 + b0)   [F, T]   mm1: lhsT=w0 chunk [128,128], rhs=xT chunk
  y  = hT.T @ w1[e]     [T, D]   mm2: lhsT=hT chunk [128,128], rhs=w1 chunk
Host combines: out[idx_e] += r_e * y_e rows; b1 folded in via routing @ b1.

All matmuls run as float32r (tf32-like, ~1e-4 rel err, full PE rate:
1 cycle/row warm). PE is the bottleneck; everything else overlaps.
"""

import numpy as np

import concourse.bacc as bacc
import concourse.mybir as mybir
from concourse.tile import TileContext
from concourse.bass_utils import run_bass_kernel_spmd

F32 = mybir.dt.float32
F32R = mybir.dt.float32r

N, D, E, F = 4096, 1024, 8, 2048
P = 128
KD = D // P            # 8  k-tiles for mm1
KF = F // P            # 16 k-tiles for mm2
T_CHUNKS = [256, 384, 512]   # mm1 moving-dim chunks (>=256 keeps fp32r full rate)
TCH = 384              # token pad granularity
D_CHUNKS = [512, 512]        # mm2 moving-dim chunks (sum = D)
assert sum(D_CHUNKS) == D
KH = KF // 2                 # w1 streamed in (dc, k-half) tiles

_cache: dict[int, object] = {}


def build_program(T: int):
    """Bass program for one expert shard with T padded tokens."""
    assert T % TCH == 0 and T % P == 0
    TO = T // P
    # graduated chunk sizes: small first chunk -> earlier PE start
    if T == 1152:
        t_chunks = list(T_CHUNKS)
    else:
        t_chunks = [TCH] * (T // TCH)
    NTC = len(t_chunks)
    t_offs = [0, *np.cumsum(t_chunks).tolist()]

    nc = bacc.Bacc("TRN2", target_bir_lowering=False, debug=False)
    xT = nc.dram_tensor("xT", [D, T], F32R, kind="ExternalInput")
    w0 = nc.dram_tensor("w0", [D, F], F32R, kind="ExternalInput")
    w1 = nc.dram_tensor("w1", [F, D], F32R, kind="ExternalInput")
    # b0 comes pre-arranged [128, KF] on the host so the DMA is contiguous
    # 64B runs per partition (a [F]-strided load is 2048 4-byte descriptors
    # that clog the ring for ~10us)
    b0 = nc.dram_tensor("b0", [P, KF], F32, kind="ExternalInput")
    y = nc.dram_tensor("y", [T, D], F32, kind="ExternalOutput")

    xT_r = xT.rearrange("(ko p) t -> p ko t", p=P)
    w0_r = w0.rearrange("(ko p) f -> p ko f", p=P)
    w1_r = w1.rearrange("(ko p) d -> p ko d", p=P)

    with TileContext(nc) as tc:
        with tc.tile_pool(name="const", bufs=1) as const, \
             tc.tile_pool(name="xpool", bufs=1) as xpool, \
             tc.tile_pool(name="hpool", bufs=1) as hpool, \
             tc.tile_pool(name="w0pool", bufs=8) as w0pool, \
             tc.tile_pool(name="w1pool", bufs=4) as w1pool, \
             tc.tile_pool(name="ypool", bufs=1) as ypool, \
             tc.tile_pool(name="psum", bufs=8, space="PSUM") as psum:

            # x resident in SBUF at the head of the scalar ring (before any
            # gelu ACT and before b0 — few big DMAs: many small ones hit
            # HWDGE semaphore-reuse round-trips); w0 owns the sync ring
            x_sb = []
            for t in range(NTC):
                xt = xpool.tile([P, KD, t_chunks[t]], F32R, tag=f"x{t}",
                                name=f"x{t}")
                nc.scalar.dma_start(xt[:], xT_r[:, :, t_offs[t]:t_offs[t + 1]])
                x_sb.append(xt)

            b0_sb = const.tile([P, KF], F32)
            nc.scalar.dma_start(b0_sb[:], b0[:, :])

            # hT = gelu(x @ w0 + b0), laid out [F-part, T-free], fp32r
            h_sb = hpool.tile([P, KF, T], F32R)

            # w1 (dc, k-half) tiles; DMAs dripped into the scalar stream
            # mid-phase-1 (behind gelu ACTs) so they can't steal bandwidth
            # from the ramp. The last shares a slot with the first and is
            # issued in phase 2 once the slot frees.
            w1_sb = {}
            for dc in range(len(D_CHUNKS)):
                for kh in range(2):
                    w1_sb[dc, kh] = w1pool.tile([P, KH, 512], F32R, tag="w1",
                                                name=f"w1_{dc}_{kh}")

            def load_w1(engine, dc, kh):
                engine.dma_start(
                    w1_sb[dc, kh][:],
                    w1_r[:, kh * KH:(kh + 1) * KH, dc * 512:(dc + 1) * 512])

            # ---- phase 1: mm1 + gelu ----
            # (fo; t; k): the first psum group needs only xT chunk t0, so the
            # PE starts while t1/t2 are still arriving
            # skew-2 order: chunk t defers 2 fo behind chunk t-1, so the PE
            # always has t0 work while later xT chunks arrive (w0 bufs=8
            # keeps ~2-3 groups of slot slack over the skew)
            pairs = sorted(((f, t) for f in range(KF) for t in range(NTC)),
                           key=lambda ft: (ft[0] + 2 * ft[1], ft[1]))
            gate_tile = const.tile([P, 1], F32R, name="gate_tile")
            w0_tiles = {}
            for gi, (fo, t) in enumerate(pairs):
                w0_sb = w0_tiles.get(fo)
                if w0_sb is None:
                    if fo == 3:
                        # tiny SBUF->SBUF dummy reading xT chunk 0: FIFO holds
                        # the sync ring here until t0 lands, so deep w0
                        # prefetch can't bandwidth-starve the first chunk
                        nc.sync.dma_start(gate_tile[:], x_sb[0][:, 0, 0:1])
                    w0_sb = w0_tiles[fo] = w0pool.tile([P, KD, P], F32R,
                                                       tag="w0", name=f"w0_{fo}")
                    nc.sync.dma_start(w0_sb[:], w0_r[:, :, fo * P:(fo + 1) * P])
                ps = psum.tile([P, 512], F32, tag="ps",
                               name=f"ps1_{fo}_{t}")[:, :t_chunks[t]]
                for k in range(KD):
                    nc.tensor.matmul(ps, w0_sb[:, k], x_sb[t][:, k],
                                     start=(k == 0), stop=(k == KD - 1))
                nc.scalar.activation(h_sb[:, fo, t_offs[t]:t_offs[t + 1]], ps,
                                     mybir.ActivationFunctionType.Gelu,
                                     bias=b0_sb[:, fo:fo + 1])
                # drip the w1 loads into the scalar stream mid-phase-1 so
                # they don't compete with xT/w0 during the ramp
                if t == 2 and fo in (6, 8, 10, 12):
                    dc, kh = [(0, 0), (0, 1), (1, 0), (1, 1)][(fo - 6) // 2]
                    load_w1(nc.scalar, dc, kh)

            # ---- phase 2: mm2 ----
            for dc, DCH in enumerate(D_CHUNKS):
                for to in range(TO):
                    ps2 = psum.tile([P, 512], F32, tag="ps",
                                    name=f"ps2_{dc}_{to}")
                    for k in range(KF):
                        nc.tensor.matmul(ps2, h_sb[:, k, to * P:(to + 1) * P],
                                         w1_sb[dc, k // KH][:, k % KH],
                                         start=(k == 0), stop=(k == KF - 1))
                    y_sb = ypool.tile([P, 512], F32, tag="y",
                                      name=f"y_{dc}_{to}")
                    nc.vector.tensor_copy(y_sb[:], ps2)
                    nc.sync.dma_start(
                        y[to * P:(to + 1) * P, dc * 512:(dc + 1) * 512], y_sb[:])

    nc.compile()
    return nc


def kernel(x, routing_tensor, w0, b0, w1, b1):
    x = np.ascontiguousarray(np.asarray(x, dtype=np.float32))
    routing = np.asarray(routing_tensor, dtype=np.float32)
    w0 = np.ascontiguousarray(np.asarray(w0, dtype=np.float32))
    b0 = np.asarray(b0, dtype=np.float32)
    w1 = np.ascontiguousarray(np.asarray(w1, dtype=np.float32))
    b1 = np.asarray(b1, dtype=np.float32)

    idx = [np.nonzero(routing[:, e])[0] for e in range(E)]
    cnt = [len(i) for i in idx]
    T = max(TCH, -(-max(cnt) // TCH) * TCH)

    nc = _cache.get(T)
    if nc is None:
        nc = _cache[T] = build_program(T)

    in_maps = []
    for e in range(E):
        xTe = np.zeros((D, T), dtype=np.float32)
        xTe[:, :cnt[e]] = x[idx[e]].T
        b0e = np.ascontiguousarray(b0[e, 0].reshape(KF, P).T)
        in_maps.append({"xT": xTe, "w0": w0[e], "w1": w1[e], "b0": b0e})

    res = run_bass_kernel_spmd(nc, in_maps, core_ids=list(range(E)))

    # combine: out = sum_e r_e * (y_e + b1_e)
    out = routing @ b1[:, 0, :]
    for e in range(E):
        r = routing[idx[e], e:e + 1]
        out[idx[e]] += r * res.results[e]["y"][:cnt[e]]
    return out.astype(np.float32)


# revision 25
# speedup vs baseline: 1.1042x; 1.1042x over previous
"""MoE BatchedExperts kernel for 8 trn2 NeuronCores.

Strategy: expert parallelism with host-side top-k dispatch. Each token has
exactly TOP_K nonzero routing weights, so core e only processes the tokens
routed to expert e (~N*K/E of them) instead of all N — 4x less compute than
the dense reference formulation, identical math (zero-score tokens
contribute zero).

Per core e (tokens gathered+transposed on host to xT [D, T]):
  hT = gelu(mm1 + b0)   [F, T]   mm1: lhsT=w0 chunk [128,128], rhs=xT chunk
  y  = hT.T @ w1[e]     [T, D]   mm2: lhsT=hT chunk [128,128], rhs=w1 chunk
Host combines: out[idx_e] += r_e * y_e rows; b1 folded in via routing @ b1.

All matmuls run as float32r (tf32-like, ~1e-4 rel err, full PE rate:
1 cycle/row warm). PE is the bottleneck; everything else overlaps.
"""

import numpy as np

import concourse.bacc as bacc
import concourse.mybir as mybir
from concourse.tile import TileContext
from concourse.bass_utils import run_bass_kernel_spmd

F32 = mybir.dt.float32
F32R = mybir.dt.float32r

N, D, E, F = 4096, 1024, 8, 2048
P = 128
KD = D // P            # 8  k-tiles for mm1
KF = F // P            # 16 k-tiles for mm2
T_CHUNKS = [256, 384, 512]   # mm1 moving-dim chunks (>=256 keeps fp32r full rate)
TCH = 384              # token pad granularity
D_CHUNKS = [512, 512]        # mm2 moving-dim chunks (sum = D)
assert sum(D_CHUNKS) == D
KH = KF // 2                 # w1 streamed in (dc, k-half) tiles

_cache: dict[int, object] = {}


def build_program(T: int):
    """Bass program for one expert shard with T padded tokens."""
    assert T % TCH == 0 and T % P == 0
    TO = T // P
    # graduated chunk sizes: small first chunk -> earlier PE start
    if T == 1152:
        t_chunks = list(T_CHUNKS)
    else:
        t_chunks = [TCH] * (T // TCH)
    NTC = len(t_chunks)
    t_offs = [0, *np.cumsum(t_chunks).tolist()]

    nc = bacc.Bacc("TRN2", target_bir_lowering=False, debug=False)
    xT = nc.dram_tensor("xT", [D, T], F32R, kind="ExternalInput")
    w0 = nc.dram_tensor("w0", [D, F], F32R, kind="ExternalInput")
    w1 = nc.dram_tensor("w1", [F, D], F32R, kind="ExternalInput")
    # b0 comes pre-arranged [128, KF] on the host so the DMA is contiguous
    # 64B runs per partition (a [F]-strided load is 2048 4-byte descriptors
    # that clog the ring for ~10us)
    b0 = nc.dram_tensor("b0", [P, KF], F32, kind="ExternalInput")
    y = nc.dram_tensor("y", [T, D], F32, kind="ExternalOutput")

    xT_r = xT.rearrange("(ko p) t -> p ko t", p=P)
    w0_r = w0.rearrange("(ko p) f -> p ko f", p=P)
    w1_r = w1.rearrange("(ko p) d -> p ko d", p=P)

    with TileContext(nc) as tc:
        with tc.tile_pool(name="const", bufs=1) as const, \
             tc.tile_pool(name="xpool", bufs=1) as xpool, \
             tc.tile_pool(name="hpool", bufs=1) as hpool, \
             tc.tile_pool(name="w0pool", bufs=5) as w0pool, \
             tc.tile_pool(name="w1pool", bufs=4) as w1pool, \
             tc.tile_pool(name="ypool", bufs=3) as ypool, \
             tc.tile_pool(name="psum", bufs=8, space="PSUM") as psum:

            # x resident in SBUF at the head of the scalar ring (before any
            # gelu ACT and before b0 — few big DMAs: many small ones hit
            # HWDGE semaphore-reuse round-trips); w0 owns the sync ring
            x_sb = []
            for t in range(NTC):
                xt = xpool.tile([P, KD, t_chunks[t]], F32R, tag=f"x{t}",
                                name=f"x{t}")
                nc.scalar.dma_start(xt[:], xT_r[:, :, t_offs[t]:t_offs[t + 1]])
                x_sb.append(xt)

            b0_sb = const.tile([P, KF], F32)
            nc.scalar.dma_start(b0_sb[:], b0[:, :])

            # hT = gelu(x @ w0 + b0), laid out [F-part, T-free], fp32r
            h_sb = hpool.tile([P, KF, T], F32R)

            # w1 (dc, k-half) tiles; DMAs dripped into the scalar stream
            # mid-phase-1 (behind gelu ACTs) so they can't steal bandwidth
            # from the ramp. The last shares a slot with the first and is
            # issued in phase 2 once the slot frees.
            w1_sb = {}
            for dc in range(len(D_CHUNKS)):
                for kh in range(2):
                    w1_sb[dc, kh] = w1pool.tile([P, KH, 512], F32R, tag="w1",
                                                name=f"w1_{dc}_{kh}")

            def load_w1(engine, dc, kh):
                engine.dma_start(
                    w1_sb[dc, kh][:],
                    w1_r[:, kh * KH:(kh + 1) * KH, dc * 512:(dc + 1) * 512])

            # ---- phase 1: mm1 + gelu ----
            # (fo; t; k): the first psum group needs only xT chunk t0, so the
            # PE starts while t1/t2 are still arriving
            # first three fo interleaved t-major: 3x more PE work available
            # per arrived xT chunk during the ramp
            pairs = [(f, t) for t in range(NTC) for f in range(3)]
            pairs += [(f, t) for f in range(3, KF) for t in range(NTC)]
            gate_tile = const.tile([P, 1], F32R, name="gate_tile")
            w0_tiles = {}
            for gi, (fo, t) in enumerate(pairs):
                w0_sb = w0_tiles.get(fo)
                if w0_sb is None:
                    if fo == 3:
                        # tiny SBUF->SBUF dummy reading xT chunk 0: FIFO holds
                        # the sync ring here until t0 lands, so deep w0
                        # prefetch can't bandwidth-starve the first chunk
                        nc.sync.dma_start(gate_tile[:], x_sb[0][:, 0, 0:1])
                    w0_sb = w0_tiles[fo] = w0pool.tile([P, KD, P], F32R,
                                                       tag="w0", name=f"w0_{fo}")
                    nc.sync.dma_start(w0_sb[:], w0_r[:, :, fo * P:(fo + 1) * P])
                ps = psum.tile([P, 512], F32, tag="ps",
                               name=f"ps1_{fo}_{t}")[:, :t_chunks[t]]
                for k in range(KD):
                    nc.tensor.matmul(ps, w0_sb[:, k], x_sb[t][:, k],
                                     start=(k == 0), stop=(k == KD - 1))
                nc.scalar.activation(h_sb[:, fo, t_offs[t]:t_offs[t + 1]], ps,
                                     mybir.ActivationFunctionType.Gelu,
                                     bias=b0_sb[:, fo:fo + 1])
                # drip the w1 loads into the scalar stream mid-phase-1 so
                # they don't compete with xT/w0 during the ramp
                if t == 2 and fo in (6, 8, 10, 12):
                    dc, kh = [(0, 0), (0, 1), (1, 0), (1, 1)][(fo - 6) // 2]
                    load_w1(nc.scalar, dc, kh)

            # ---- phase 2: mm2 ----
            for dc, DCH in enumerate(D_CHUNKS):
                for to in range(TO):
                    ps2 = psum.tile([P, 512], F32, tag="ps",
                                    name=f"ps2_{dc}_{to}")
                    for k in range(KF):
                        nc.tensor.matmul(ps2, h_sb[:, k, to * P:(to + 1) * P],
                                         w1_sb[dc, k // KH][:, k % KH],
                                         start=(k == 0), stop=(k == KF - 1))
                    y_sb = ypool.tile([P, 512], F32, tag="y",
                                      name=f"y_{dc}_{to}")
                    nc.vector.tensor_copy(y_sb[:], ps2)
                    nc.sync.dma_start(
                        y[to * P:(to + 1) * P, dc * 512:(dc + 1) * 512], y_sb[:])

    nc.compile()
    return nc


def kernel(x, routing_tensor, w0, b0, w1, b1):
    x = np.ascontiguousarray(np.asarray(x, dtype=np.float32))
    routing = np.asarray(routing_tensor, dtype=np.float32)
    w0 = np.ascontiguousarray(np.asarray(w0, dtype=np.float32))
    b0 = np.asarray(b0, dtype=np.float32)
    w1 = np.ascontiguousarray(np.asarray(w1, dtype=np.float32))
    b1 = np.asarray(b1, dtype=np.float32)

    idx = [np.nonzero(routing[:, e])[0] for e in range(E)]
    cnt = [len(i) for i in idx]
    T = max(TCH, -(-max(cnt) // TCH) * TCH)

    nc = _cache.get(T)
    if nc is None:
        nc = _cache[T] = build_program(T)

    in_maps = []
    for e in range(E):
        xTe = np.zeros((D, T), dtype=np.float32)
        xTe[:, :cnt[e]] = x[idx[e]].T
        b0e = np.ascontiguousarray(b0[e, 0].reshape(KF, P).T)
        in_maps.append({"xT": xTe, "w0": w0[e], "w1": w1[e], "b0": b0e})

    res = run_bass_kernel_spmd(nc, in_maps, core_ids=list(range(E)))

    # combine: out = sum_e r_e * (y_e + b1_e)
    out = routing @ b1[:, 0, :]
    for e in range(E):
        r = routing[idx[e], e:e + 1]
        out[idx[e]] += r * res.results[e]["y"][:cnt[e]]
    return out.astype(np.float32)
